# revision 21
# baseline (speedup 1.0000x reference)
"""Depth-warping layer for Trainium2 (Bass/Tile), 8-core data-parallel.

Strategy (v2)
-------------
Pure data parallelism over batch: each of 8 NeuronCores processes 2 of
the 16 images end to end (no collectives).

Per image:
  Phase A: build a "quad table" J2[(H+1) x (W+1), 4] in device DRAM,
    J2[r, c] = (I[rA,cA], I[rB,cA], I[rA,cB], I[rB,cB]) with
    rA=r-1, rB=min(r,H-1), cA=c-1, cB=min(c,W-1) (rows r>=1, cols
    1..W; row 0 is zeros and cols 0/W carry pairwise-equal taps so the
    clipped-border bilinear weights cancel exactly, matching the
    reference's clip semantics). I = d1_calc = W2z + d2*(affine in x,y).
    Built fully in SBUF per 128-row tile (strided interleave copies,
    row+1 tap via an SBUF partition-shift DMA), then ONE contiguous
    2.6MB DMA store per tile — no strided DRAM writes.
  Phase B: coordinate math in f32 only (floor via the 1.5*2^23
    round-to-nearest trick, so no dependence on the f32->i32 convert
    rounding mode), one i32 convert for the flattened gather index,
    then per-pixel 16B-quad gathers via single-offset-per-partition
    indirect DMA (128 descriptors/instruction — the only form the
    SWDGE vector-DGE executes correctly; multi-offset APs scramble),
    fully unrolled (no tc.For_i: its per-iteration all-engine barrier
    +dge-drain costs ~15%) over 4 SWDGE queues, and a factored 9-op
    bilinear combine.

The gather is the wall: one 128-descriptor Pool instruction per 128
pixels at ~1.1us fixed SWDGE cost each, so everything else is arranged
to hide completely under the Pool engine's instruction stream.

Host does only the per-batch 3x3 matrix algebra and ships per-batch
row/column/scalar coefficient tensors (the NEFF is shared by all cores,
so per-batch constants arrive as data).
"""

import numpy as np

import concourse.bass as bass
import concourse.bacc as bacc
import concourse.mybir as mybir
from concourse.tile import TileContext
from concourse import bass_utils

B, H, W = 16, 1024, 1280
NCORES = 8
BPC = B // NCORES          # batches per core
HP = H + 1                 # J2 rows
WP = W + 1                 # J2 cols
NTILES = H // 128
WT = 640                   # phase-B column tile (2 per row tile)
NCT = W // WT
UNROLL = 64                # gathers per For_i iteration

F32 = mybir.dt.float32
I32 = mybir.dt.int32
OP = mybir.AluOpType
ACTF = mybir.ActivationFunctionType

MAGIC = 12582912.0         # 1.5 * 2^23: forces round-to-int in f32 add


def _build_bass():
    nc = bacc.Bacc(target_bir_lowering=False, num_swdge_queues=4)

    d1 = nc.dram_tensor("d1", [BPC, H, W], F32, kind="ExternalInput")
    d2 = nc.dram_tensor("d2", [BPC, H, W], F32, kind="ExternalInput")
    # Replicated row-planes [BPC, 128, W] (DVE/ACT cannot partition-
    # broadcast, so host replicates). rowA/rowB are NEGATED on host
    # (-M00*x, -M10*x) because q approximates -1/z2.
    rowA = nc.dram_tensor("rowA", [BPC, 128, W], F32, kind="ExternalInput")
    rowB = nc.dram_tensor("rowB", [BPC, 128, W], F32, kind="ExternalInput")
    rowC = nc.dram_tensor("rowC", [BPC, 128, W], F32, kind="ExternalInput")
    rowG = nc.dram_tensor("rowG", [BPC, 128, W], F32, kind="ExternalInput")
    # Per-tile per-partition columns [BPC, 128, NTILES]; constants folded:
    # colA = -(M01*y + M02), colB = -(M11*y + M12), colC = M21*y + M22,
    # colG = M2_21*y + M2_22.
    colA = nc.dram_tensor("colA", [BPC, 128, NTILES], F32, kind="ExternalInput")
    colB = nc.dram_tensor("colB", [BPC, 128, NTILES], F32, kind="ExternalInput")
    colC = nc.dram_tensor("colC", [BPC, 128, NTILES], F32, kind="ExternalInput")
    colG = nc.dram_tensor("colG", [BPC, 128, NTILES], F32, kind="ExternalInput")
    colGB = nc.dram_tensor("colGB", [BPC, 128, NTILES], F32, kind="ExternalInput")
    # Per-batch scalars [BPC, 128, 8]:
    # (-Wv0, -Wv1, +Wv2, +W2z, -M02, -M12, +M22, +M2_22)
    sc = nc.dram_tensor("sc", [BPC, 128, 8], F32, kind="ExternalInput")
    out = nc.dram_tensor("out", [BPC, H, W], F32, kind="ExternalOutput")

    with TileContext(nc) as tc:
        with tc.tile_pool(name="dram", bufs=2, space="DRAM") as dpool, \
             tc.tile_pool(name="cst", bufs=1) as cpool, \
             tc.tile_pool(name="io", bufs=2) as iop, \
             tc.tile_pool(name="ph_a", bufs=2) as ap_, \
             tc.tile_pool(name="tmp", bufs=1) as tp, \
             tc.tile_pool(name="pipe", bufs=2) as pp, \
             tc.tile_pool(name="gat", bufs=2) as gp:

            # zeros for J2 row 0 (4 partitions x WP f32 = one 20.5KB row)
            zrow = cpool.tile([4, WP], F32)
            nc.vector.memset(zrow[:], 0.0)

            for lb in range(BPC):
                J2 = dpool.tile([HP, WP, 4], F32, tag="J2")
                J2flat = J2[:].rearrange("a b c -> (a b) c")

                # ---- per-batch coefficient loads ----
                sc_t = cpool.tile([128, 8], F32, tag="sc")
                nc.sync.dma_start(out=sc_t[:], in_=sc[lb])
                rowA_t = cpool.tile([128, W], F32, tag="rowA")
                rowB_t = cpool.tile([128, W], F32, tag="rowB")
                rowC_t = cpool.tile([128, W], F32, tag="rowC")
                rowG_t = cpool.tile([128, W], F32, tag="rowG")
                nc.sync.dma_start(out=rowA_t[:], in_=rowA[lb])
                nc.sync.dma_start(out=rowB_t[:], in_=rowB[lb])
                nc.sync.dma_start(out=rowC_t[:], in_=rowC[lb])
                nc.sync.dma_start(out=rowG_t[:], in_=rowG[lb])
                colA_t = cpool.tile([128, NTILES], F32, tag="colA")
                colB_t = cpool.tile([128, NTILES], F32, tag="colB")
                colC_t = cpool.tile([128, NTILES], F32, tag="colC")
                colG_t = cpool.tile([128, NTILES], F32, tag="colG")
                colGB_t = cpool.tile([128, NTILES], F32, tag="colGB")
                nc.sync.dma_start(out=colA_t[:], in_=colA[lb])
                nc.sync.dma_start(out=colB_t[:], in_=colB[lb])
                nc.sync.dma_start(out=colC_t[:], in_=colC[lb])
                nc.sync.dma_start(out=colG_t[:], in_=colG[lb])
                nc.sync.dma_start(out=colGB_t[:], in_=colGB[lb])

                # ---- Phase A: build J2 ----
                # J2 row 0 = zeros (border row; weights cancel exactly)
                nc.sync.dma_start(out=J2[0:1, :, :], in_=zrow[:])

                def phase_a(t):
                    """Build J2 rows [128t+1, 128t+129): A-tap = image row
                    128t+p, B-tap = image row min(128t+p+1, H-1)."""
                    y0 = 128 * t
                    d2a = iop.tile([128, W], F32, tag="d2a")
                    d2b = iop.tile([128, W], F32, tag="d2b")
                    nc.sync.dma_start(out=d2a[:], in_=d2[lb, y0:y0 + 128, :])
                    if t < NTILES - 1:
                        nc.sync.dma_start(out=d2b[:], in_=d2[lb, y0 + 1:y0 + 129, :])
                    else:
                        nc.sync.dma_start(out=d2b[0:127, :], in_=d2[lb, y0 + 1:y0 + 128, :])
                        nc.sync.dma_start(out=d2b[127:128, :], in_=d2[lb, H - 1:H, :])
                    gA = ap_.tile([128, W], F32, tag="gA", bufs=1)
                    gB = ap_.tile([128, W], F32, tag="gB", bufs=1)
                    nc.scalar.activation(out=gA[:], in_=rowG_t[:], func=ACTF.Identity,
                                         bias=colG_t[:, t:t + 1], scale=1.0)
                    nc.scalar.activation(out=gA[:], in_=gA[:], func=ACTF.Identity,
                                         bias=sc_t[:, 7:8], scale=1.0)
                    nc.scalar.activation(out=gB[:], in_=rowG_t[:], func=ACTF.Identity,
                                         bias=colGB_t[:, t:t + 1], scale=1.0)
                    nc.scalar.activation(out=gB[:], in_=gB[:], func=ACTF.Identity,
                                         bias=sc_t[:, 7:8], scale=1.0)
                    dcA = ap_.tile([128, W], F32, tag="d1cA", bufs=1)
                    dcB = ap_.tile([128, W], F32, tag="d1cB", bufs=1)
                    nc.vector.tensor_tensor(out=dcA[:], in0=d2a[:], in1=gA[:], op=OP.mult)
                    nc.vector.tensor_tensor(out=dcB[:], in0=d2b[:], in1=gB[:], op=OP.mult)
                    nc.scalar.activation(out=dcA[:], in_=dcA[:], func=ACTF.Identity,
                                         bias=sc_t[:, 3:4], scale=1.0)
                    nc.scalar.activation(out=dcB[:], in_=dcB[:], func=ACTF.Identity,
                                         bias=sc_t[:, 3:4], scale=1.0)
                    jq = ap_.tile([128, WP, 4], F32, tag="jq", bufs=1)
                    # cA(c) = c-1 for c=1..W ; cB(c) = c for c=0..W-1
                    nc.vector.tensor_copy(out=jq[:, 1:WP, 0], in_=dcA[:, 0:W])
                    nc.scalar.copy(out=jq[:, 1:WP, 1], in_=dcB[:, 0:W])
                    nc.vector.tensor_copy(out=jq[:, 0:W, 2], in_=dcA[:, 0:W])
                    nc.scalar.copy(out=jq[:, 0:W, 3], in_=dcB[:, 0:W])
                    # border columns: c=0 taps 0/1 (cA=0), c=W taps 2/3 (cB=W-1)
                    nc.vector.tensor_copy(out=jq[:, 0:1, 0], in_=dcA[:, 0:1])
                    nc.vector.tensor_copy(out=jq[:, 0:1, 1], in_=dcB[:, 0:1])
                    nc.vector.tensor_copy(out=jq[:, WP - 1:WP, 2], in_=dcA[:, W - 1:W])
                    nc.vector.tensor_copy(out=jq[:, WP - 1:WP, 3], in_=dcB[:, W - 1:W])
                    nc.sync.dma_start(out=J2[y0 + 1:y0 + 129, :, :], in_=jq[:])

                for t in range(NTILES):
                    phase_a(t)

                # ---- Phase B ----
                def ts(dst, in0, s1, s2, o0, o1=None):
                    nc.vector.tensor_scalar(out=dst, in0=in0, scalar1=s1,
                                            scalar2=s2, op0=o0,
                                            **({"op1": o1} if o1 is not None else {}))

                pending = None  # (weights..., gq) awaiting combine

                def coords_and_gather(rt, ct, z1):
                    y0 = 128 * rt
                    c0 = WT * ct
                    z1v = z1[:, c0:c0 + WT]
                    # affine terms (ACT: row + per-partition col const)
                    At = tp.tile([128, WT], F32, tag="fa")
                    Bt = tp.tile([128, WT], F32, tag="fb")
                    Ct = tp.tile([128, WT], F32, tag="fc")
                    nc.scalar.activation(out=At[:], in_=rowA_t[:, c0:c0 + WT],
                                         func=ACTF.Identity, bias=colA_t[:, rt:rt + 1], scale=1.0)
                    nc.scalar.activation(out=At[:], in_=At[:],
                                         func=ACTF.Identity, bias=sc_t[:, 4:5], scale=1.0)
                    nc.scalar.activation(out=Bt[:], in_=rowB_t[:, c0:c0 + WT],
                                         func=ACTF.Identity, bias=colB_t[:, rt:rt + 1], scale=1.0)
                    nc.scalar.activation(out=Bt[:], in_=Bt[:],
                                         func=ACTF.Identity, bias=sc_t[:, 5:6], scale=1.0)
                    nc.scalar.activation(out=Ct[:], in_=rowC_t[:, c0:c0 + WT],
                                         func=ACTF.Identity, bias=colC_t[:, rt:rt + 1], scale=1.0)
                    nc.scalar.activation(out=Ct[:], in_=Ct[:],
                                         func=ACTF.Identity, bias=sc_t[:, 6:7], scale=1.0)
                    # z2 = z1*C + Wv2 ; q = -1/z2 (recip + 1 Newton, sign folded)
                    z2 = tp.tile([128, WT], F32, tag="fd")
                    nc.vector.tensor_tensor(out=z2[:], in0=z1v, in1=Ct[:], op=OP.mult)
                    nc.scalar.activation(out=z2[:], in_=z2[:], func=ACTF.Identity,
                                         bias=sc_t[:, 2:3], scale=1.0)
                    r0 = tp.tile([128, WT], F32, tag="fe")
                    nc.vector.reciprocal(out=r0[:], in_=z2[:])
                    e = tp.tile([128, WT], F32, tag="ff")
                    nc.vector.tensor_tensor(out=e[:], in0=z2[:], in1=r0[:], op=OP.mult)
                    ts(e[:], e[:], 2.0, None, OP.subtract)            # z2*r0 - 2
                    q = tp.tile([128, WT], F32, tag="fq")
                    nc.vector.tensor_tensor(out=q[:], in0=r0[:], in1=e[:], op=OP.mult)  # ~ -1/z2
                    # u2 = (z1*(-A) + (-Wv0)) * q ; v2 likewise
                    nU = tp.tile([128, WT], F32, tag="fn")
                    nc.vector.tensor_tensor(out=nU[:], in0=z1v, in1=At[:], op=OP.mult)
                    nc.scalar.activation(out=nU[:], in_=nU[:], func=ACTF.Identity,
                                         bias=sc_t[:, 0:1], scale=1.0)
                    u2 = tp.tile([128, WT], F32, tag="fu")
                    nc.vector.tensor_tensor(out=u2[:], in0=nU[:], in1=q[:], op=OP.mult)
                    # Markstein correction: e = z2*u2' + nU ; u2 = u2' + e*q
                    # (keeps u2 within 0.5ulp of the reference's IEEE divide,
                    # which matters at the clip-edge discontinuities)
                    nc.vector.tensor_tensor(out=e[:], in0=z2[:], in1=u2[:], op=OP.mult)
                    nc.vector.tensor_tensor(out=e[:], in0=e[:], in1=nU[:], op=OP.add)
                    nc.vector.tensor_tensor(out=e[:], in0=e[:], in1=q[:], op=OP.mult)
                    nc.vector.tensor_tensor(out=u2[:], in0=u2[:], in1=e[:], op=OP.add)
                    nc.vector.tensor_tensor(out=nU[:], in0=z1v, in1=Bt[:], op=OP.mult)
                    nc.scalar.activation(out=nU[:], in_=nU[:], func=ACTF.Identity,
                                         bias=sc_t[:, 1:2], scale=1.0)
                    v2 = tp.tile([128, WT], F32, tag="fv")
                    nc.vector.tensor_tensor(out=v2[:], in0=nU[:], in1=q[:], op=OP.mult)
                    nc.vector.tensor_tensor(out=e[:], in0=z2[:], in1=v2[:], op=OP.mult)
                    nc.vector.tensor_tensor(out=e[:], in0=e[:], in1=nU[:], op=OP.add)
                    nc.vector.tensor_tensor(out=e[:], in0=e[:], in1=q[:], op=OP.mult)
                    nc.vector.tensor_tensor(out=v2[:], in0=v2[:], in1=e[:], op=OP.add)

                    # exact floor in f32: t = RNE(x) via magic add/sub (two
                    # separate instructions so the intermediate rounds to
                    # f32), then t -= (t > x). No convert-mode dependence.
                    xf = tp.tile([128, WT], F32, tag="xf")
                    yf = tp.tile([128, WT], F32, tag="yf")
                    gt = tp.tile([128, WT], F32, tag="gtf")
                    ts(xf[:], u2[:], MAGIC, None, OP.add)
                    ts(xf[:], xf[:], MAGIC, None, OP.subtract)
                    nc.vector.tensor_tensor(out=gt[:], in0=xf[:], in1=u2[:], op=OP.is_gt)
                    nc.vector.tensor_tensor(out=xf[:], in0=xf[:], in1=gt[:], op=OP.subtract)
                    ts(yf[:], v2[:], MAGIC, None, OP.add)
                    ts(yf[:], yf[:], MAGIC, None, OP.subtract)
                    nc.vector.tensor_tensor(out=gt[:], in0=yf[:], in1=v2[:], op=OP.is_gt)
                    nc.vector.tensor_tensor(out=yf[:], in0=yf[:], in1=gt[:], op=OP.subtract)
                    # clips (float domain; all values integer-exact)
                    x1p = tp.tile([128, WT], F32, tag="x1p")
                    ccf = tp.tile([128, WT], F32, tag="ccf")
                    y1p = tp.tile([128, WT], F32, tag="y1p")
                    rrf = tp.tile([128, WT], F32, tag="rrf")
                    ts(x1p[:], xf[:], 1.0, 0.0, OP.add, OP.max)
                    ts(ccf[:], x1p[:], float(W), None, OP.min)
                    ts(y1p[:], yf[:], 1.0, 0.0, OP.add, OP.max)
                    ts(rrf[:], y1p[:], float(H), None, OP.min)
                    # weights tiles (double-buffered across the pipeline)
                    wxa = pp.tile([128, WT], F32, tag="wxa")
                    wxc = pp.tile([128, WT], F32, tag="wxc")
                    wya = pp.tile([128, WT], F32, tag="wya")
                    wyb = pp.tile([128, WT], F32, tag="wyb")
                    # x0f = clip(xf,0,W-1), x1f = min(x1p,W-1) (reuse xf/x1p)
                    ts(xf[:], xf[:], 0.0, float(W - 1), OP.max, OP.min)
                    ts(x1p[:], x1p[:], float(W - 1), None, OP.min)
                    ts(yf[:], yf[:], 0.0, float(H - 1), OP.max, OP.min)
                    ts(y1p[:], y1p[:], float(H - 1), None, OP.min)
                    nc.vector.tensor_tensor(out=wxa[:], in0=x1p[:], in1=u2[:], op=OP.subtract)
                    nc.vector.tensor_tensor(out=wxc[:], in0=u2[:], in1=xf[:], op=OP.subtract)
                    nc.vector.tensor_tensor(out=wya[:], in0=y1p[:], in1=v2[:], op=OP.subtract)
                    nc.vector.tensor_tensor(out=wyb[:], in0=v2[:], in1=yf[:], op=OP.subtract)
                    # flat index = rr*WP + cc (exact in f32), one i32 convert
                    flatf = tp.tile([128, WT], F32, tag="flatf")
                    nc.vector.scalar_tensor_tensor(
                        out=flatf[:], in0=rrf[:], scalar=float(WP), in1=ccf[:],
                        op0=OP.mult, op1=OP.add)
                    flat = pp.tile([128, WT], I32, tag="flat")
                    nc.vector.tensor_copy(out=flat[:], in_=flatf[:])

                    # single-offset indirect gathers: 128 quads (one column
                    # of the col-tile) per instruction, fully unrolled so the
                    # Pool stream is pure INDIRECT1D (no For_i all-engine
                    # barriers / dge-drains), 4 SWDGE queues round-robin
                    gq = gp.tile([128, WT, 4], F32, tag="gq")
                    for j in range(WT):
                        inst = nc.gpsimd.indirect_dma_start(
                            out=gq[:, j, :], out_offset=None,
                            in_=J2flat,
                            in_offset=bass.IndirectOffsetOnAxis(ap=flat[:, j:j + 1], axis=0),
                        )
                        inst.ins.queue = f"qPoolDynamic{j % 4 or ''}"
                    return (rt, ct, wxa, wxc, wya, wyb, gq)

                def combine(p):
                    rt, ct, wxa, wxc, wya, wyb, gq = p
                    y0 = 128 * rt
                    c0 = WT * ct
                    px0 = tp.tile([128, WT], F32, tag="fa")
                    px1 = tp.tile([128, WT], F32, tag="fb")
                    tt = tp.tile([128, WT], F32, tag="fc")
                    nc.vector.tensor_tensor(out=px0[:], in0=wxa[:], in1=gq[:, :, 0], op=OP.mult)
                    nc.vector.tensor_tensor(out=tt[:], in0=wxc[:], in1=gq[:, :, 2], op=OP.mult)
                    nc.vector.tensor_tensor(out=px0[:], in0=px0[:], in1=tt[:], op=OP.add)
                    nc.vector.tensor_tensor(out=px1[:], in0=wxa[:], in1=gq[:, :, 1], op=OP.mult)
                    nc.vector.tensor_tensor(out=tt[:], in0=wxc[:], in1=gq[:, :, 3], op=OP.mult)
                    nc.vector.tensor_tensor(out=px1[:], in0=px1[:], in1=tt[:], op=OP.add)
                    o = iop.tile([128, WT], F32, tag="ot")
                    nc.vector.tensor_tensor(out=o[:], in0=wya[:], in1=px0[:], op=OP.mult)
                    nc.vector.tensor_tensor(out=tt[:], in0=wyb[:], in1=px1[:], op=OP.mult)
                    nc.vector.tensor_tensor(out=o[:], in0=o[:], in1=tt[:], op=OP.add)
                    nc.sync.dma_start(out=out[lb, y0:y0 + 128, c0:c0 + WT], in_=o[:])

                for rt in range(NTILES):
                    z1 = iop.tile([128, W], F32, tag="z1")
                    nc.sync.dma_start(out=z1[:], in_=d1[lb, 128 * rt:128 * rt + 128, :])
                    for ct in range(NCT):
                        cur = coords_and_gather(rt, ct, z1)
                        if pending is not None:
                            combine(pending)
                        pending = cur
                combine(pending)

    nc.finalize()
    return nc


def _host_aux(translation, rotation, intrinsic):
    """Per-batch coefficient tensors (f32, mirroring reference order)."""
    K = intrinsic.astype(np.float32)
    Kinv = np.linalg.inv(K).astype(np.float32)
    R = rotation.astype(np.float32)
    t = translation.astype(np.float32)
    nb = R.shape[0]
    temp = np.einsum('ij,bkj->bik', K, R).astype(np.float32)
    Wv = np.einsum('bij,bjk->bik', temp, -t).astype(np.float32)     # [nb,3,1]
    M = np.einsum('bij,jk->bik', temp, Kinv).astype(np.float32)     # [nb,3,3]
    W2 = np.einsum('ij,bjk->bik', K, t).astype(np.float32)
    M2 = np.einsum('bij,jk->bik', np.einsum('ij,bjk->bik', K, R), Kinv).astype(np.float32)

    x = np.arange(W, dtype=np.float32)
    y = np.arange(H, dtype=np.float32)
    ycols = y.reshape(NTILES, 128).T                                # [128, NTILES]

    def rep_row(v):     # [nb, W] -> [nb, 128, W]
        return np.repeat(v[:, None, :], 128, axis=1).astype(np.float32)

    aux = {}
    aux["rowA"] = rep_row(-(M[:, 0, 0][:, None] * x[None, :]))
    aux["rowB"] = rep_row(-(M[:, 1, 0][:, None] * x[None, :]))
    aux["rowC"] = rep_row(M[:, 2, 0][:, None] * x[None, :])
    aux["rowG"] = rep_row(M2[:, 2, 0][:, None] * x[None, :])
    aux["colA"] = -(M[:, 0, 1][:, None, None] * ycols[None])
    aux["colB"] = -(M[:, 1, 1][:, None, None] * ycols[None])
    aux["colC"] = (M[:, 2, 1][:, None, None] * ycols[None])
    aux["colG"] = (M2[:, 2, 1][:, None, None] * ycols[None])
    ycolsB = np.minimum(ycols + 1.0, float(H - 1)).astype(np.float32)
    aux["colGB"] = (M2[:, 2, 1][:, None, None] * ycolsB[None])
    sc = np.stack([-Wv[:, 0, 0], -Wv[:, 1, 0], Wv[:, 2, 0], W2[:, 2, 0],
                   -M[:, 0, 2], -M[:, 1, 2], M[:, 2, 2], M2[:, 2, 2]], axis=1)
    aux["sc"] = np.repeat(sc[:, None, :], 128, axis=1)              # [nb,128,8]
    for k in aux:
        aux[k] = np.ascontiguousarray(aux[k].astype(np.float32))
    return aux


_NC_CACHE = {}


def kernel(depth_map_1, depth_map_2, translation, rotation, intrinsic):
    d1 = np.ascontiguousarray(np.asarray(depth_map_1, dtype=np.float32)[..., 0])
    d2 = np.ascontiguousarray(np.asarray(depth_map_2, dtype=np.float32)[..., 0])
    t = np.asarray(translation, dtype=np.float32)
    R = np.asarray(rotation, dtype=np.float32)
    K = np.asarray(intrinsic, dtype=np.float32)

    if "nc" not in _NC_CACHE:
        _NC_CACHE["nc"] = _build_bass()
    nc = _NC_CACHE["nc"]

    aux = _host_aux(t, R, K)

    in_maps = []
    for c in range(NCORES):
        sl = slice(c * BPC, (c + 1) * BPC)
        m = {"d1": d1[sl], "d2": d2[sl]}
        for k, v in aux.items():
            m[k] = v[sl]
        in_maps.append(m)

    res = bass_utils.run_bass_kernel_spmd(nc, in_maps, core_ids=list(range(NCORES)))
    out = np.empty((B, H, W, 1), np.float32)
    for c in range(NCORES):
        out[c * BPC:(c + 1) * BPC, :, :, 0] = res.results[c]["out"]
    return out


# revision 25
# speedup vs baseline: 1.1675x; 1.1675x over previous
"""Depth-warping layer for Trainium2 (Bass/Tile), 8-core data-parallel.

Strategy (v2)
-------------
Pure data parallelism over batch: each of 8 NeuronCores processes 2 of
the 16 images end to end (no collectives).

Per image:
  Phase A: build a "quad table" J2[(H+1) x (W+1), 4] in device DRAM,
    J2[r, c] = (I[rA,cA], I[rB,cA], I[rA,cB], I[rB,cB]) with
    rA=r-1, rB=min(r,H-1), cA=c-1, cB=min(c,W-1) (rows r>=1, cols
    1..W; row 0 is zeros and cols 0/W carry pairwise-equal taps so the
    clipped-border bilinear weights cancel exactly, matching the
    reference's clip semantics). I = d1_calc = W2z + d2*(affine in x,y).
    Built fully in SBUF per 128-row tile (strided interleave copies,
    row+1 tap via an SBUF partition-shift DMA), then ONE contiguous
    2.6MB DMA store per tile — no strided DRAM writes.
  Phase B: coordinate math in f32 only (floor via the 1.5*2^23
    round-to-nearest trick, so no dependence on the f32->i32 convert
    rounding mode), one i32 convert for the flattened gather index,
    then per-pixel 16B-quad gathers via single-offset-per-partition
    indirect DMA (128 descriptors/instruction — the only form the
    SWDGE vector-DGE executes correctly; multi-offset APs scramble),
    fully unrolled (no tc.For_i: its per-iteration all-engine barrier
    +dge-drain costs ~15%) over 4 SWDGE queues, and a factored 9-op
    bilinear combine.

The gather is the wall: one 128-descriptor Pool instruction per 128
pixels at ~1.1us fixed SWDGE cost each, so everything else is arranged
to hide completely under the Pool engine's instruction stream.

Host does only the per-batch 3x3 matrix algebra and ships per-batch
row/column/scalar coefficient tensors (the NEFF is shared by all cores,
so per-batch constants arrive as data).
"""

import numpy as np

import concourse.bass as bass
import concourse.bacc as bacc
import concourse.mybir as mybir
from concourse.tile import TileContext
from concourse import bass_utils

B, H, W = 16, 1024, 1280
NCORES = 8
BPC = B // NCORES          # batches per core
HP = H + 1                 # J2 rows
WP = W + 1                 # J2 cols
NTILES = H // 128
WT = 640                   # phase-B column tile (2 per row tile)
NCT = W // WT
UNROLL = 64                # gathers per For_i iteration

F32 = mybir.dt.float32
I32 = mybir.dt.int32
OP = mybir.AluOpType
ACTF = mybir.ActivationFunctionType

MAGIC = 12582912.0         # 1.5 * 2^23: forces round-to-int in f32 add


def _build_bass():
    nc = bacc.Bacc(target_bir_lowering=False, num_swdge_queues=4)

    d1 = nc.dram_tensor("d1", [BPC, H, W], F32, kind="ExternalInput")
    d2 = nc.dram_tensor("d2", [BPC, H, W], F32, kind="ExternalInput")
    # Replicated row-planes [BPC, 128, W] (DVE/ACT cannot partition-
    # broadcast, so host replicates). rowA/rowB are NEGATED on host
    # (-M00*x, -M10*x) because q approximates -1/z2.
    rowA = nc.dram_tensor("rowA", [BPC, 128, W], F32, kind="ExternalInput")
    rowB = nc.dram_tensor("rowB", [BPC, 128, W], F32, kind="ExternalInput")
    rowC = nc.dram_tensor("rowC", [BPC, 128, W], F32, kind="ExternalInput")
    rowG = nc.dram_tensor("rowG", [BPC, 128, W], F32, kind="ExternalInput")
    # Per-tile per-partition columns [BPC, 128, NTILES]; constants folded:
    # colA = -(M01*y + M02), colB = -(M11*y + M12), colC = M21*y + M22,
    # colG = M2_21*y + M2_22.
    colA = nc.dram_tensor("colA", [BPC, 128, NTILES], F32, kind="ExternalInput")
    colB = nc.dram_tensor("colB", [BPC, 128, NTILES], F32, kind="ExternalInput")
    colC = nc.dram_tensor("colC", [BPC, 128, NTILES], F32, kind="ExternalInput")
    colG = nc.dram_tensor("colG", [BPC, 128, NTILES], F32, kind="ExternalInput")
    colGB = nc.dram_tensor("colGB", [BPC, 128, NTILES], F32, kind="ExternalInput")
    # Per-batch scalars [BPC, 128, 8]:
    # (-Wv0, -Wv1, +Wv2, +W2z, -M02, -M12, +M22, +M2_22)
    sc = nc.dram_tensor("sc", [BPC, 128, 8], F32, kind="ExternalInput")
    out = nc.dram_tensor("out", [BPC, H, W], F32, kind="ExternalOutput")

    with TileContext(nc) as tc:
        with tc.tile_pool(name="dram", bufs=2, space="DRAM") as dpool, \
             tc.tile_pool(name="cst", bufs=1) as cpool, \
             tc.tile_pool(name="io", bufs=2) as iop, \
             tc.tile_pool(name="ph_a", bufs=2) as ap_, \
             tc.tile_pool(name="tmp", bufs=1) as tp, \
             tc.tile_pool(name="pipe", bufs=2) as pp, \
             tc.tile_pool(name="gat", bufs=2) as gp:

            # zeros for J2 row 0 (4 partitions x WP f32 = one 20.5KB row)
            zrow = cpool.tile([4, WP], F32)
            nc.vector.memset(zrow[:], 0.0)

            for lb in range(BPC):
                # staging table: phase A writes land here (9 writer
                # instructions); ONE bulk copy then produces J2, so every
                # gather instruction carries a single J2-writer sem wait
                # instead of nine (the SEQ decodes each wait per gather).
                J2S = dpool.tile([HP, WP, 4], F32, tag="J2S")
                J2 = dpool.tile([HP, WP, 4], F32, tag="J2")
                J2flat = J2[:].rearrange("a b c -> (a b) c")

                # ---- per-batch coefficient loads ----
                sc_t = cpool.tile([128, 8], F32, tag="sc")
                nc.sync.dma_start(out=sc_t[:], in_=sc[lb])
                rowA_t = cpool.tile([128, W], F32, tag="rowA")
                rowB_t = cpool.tile([128, W], F32, tag="rowB")
                rowC_t = cpool.tile([128, W], F32, tag="rowC")
                rowG_t = cpool.tile([128, W], F32, tag="rowG")
                nc.sync.dma_start(out=rowA_t[:], in_=rowA[lb])
                nc.sync.dma_start(out=rowB_t[:], in_=rowB[lb])
                nc.sync.dma_start(out=rowC_t[:], in_=rowC[lb])
                nc.sync.dma_start(out=rowG_t[:], in_=rowG[lb])
                colA_t = cpool.tile([128, NTILES], F32, tag="colA")
                colB_t = cpool.tile([128, NTILES], F32, tag="colB")
                colC_t = cpool.tile([128, NTILES], F32, tag="colC")
                colG_t = cpool.tile([128, NTILES], F32, tag="colG")
                colGB_t = cpool.tile([128, NTILES], F32, tag="colGB")
                nc.sync.dma_start(out=colA_t[:], in_=colA[lb])
                nc.sync.dma_start(out=colB_t[:], in_=colB[lb])
                nc.sync.dma_start(out=colC_t[:], in_=colC[lb])
                nc.sync.dma_start(out=colG_t[:], in_=colG[lb])
                nc.sync.dma_start(out=colGB_t[:], in_=colGB[lb])

                # ---- Phase A: build J2 (into J2S) ----
                # row 0 = zeros (border row; weights cancel exactly)
                nc.sync.dma_start(out=J2S[0:1, :, :], in_=zrow[:])

                def phase_a(t):
                    """Build J2 rows [128t+1, 128t+129): A-tap = image row
                    128t+p, B-tap = image row min(128t+p+1, H-1)."""
                    y0 = 128 * t
                    d2a = iop.tile([128, W], F32, tag="d2a")
                    d2b = iop.tile([128, W], F32, tag="d2b")
                    nc.sync.dma_start(out=d2a[:], in_=d2[lb, y0:y0 + 128, :])
                    if t < NTILES - 1:
                        nc.sync.dma_start(out=d2b[:], in_=d2[lb, y0 + 1:y0 + 129, :])
                    else:
                        nc.sync.dma_start(out=d2b[0:127, :], in_=d2[lb, y0 + 1:y0 + 128, :])
                        nc.sync.dma_start(out=d2b[127:128, :], in_=d2[lb, H - 1:H, :])
                    gA = ap_.tile([128, W], F32, tag="gA", bufs=1)
                    gB = ap_.tile([128, W], F32, tag="gB", bufs=1)
                    nc.scalar.activation(out=gA[:], in_=rowG_t[:], func=ACTF.Identity,
                                         bias=colG_t[:, t:t + 1], scale=1.0)
                    nc.scalar.activation(out=gA[:], in_=gA[:], func=ACTF.Identity,
                                         bias=sc_t[:, 7:8], scale=1.0)
                    nc.scalar.activation(out=gB[:], in_=rowG_t[:], func=ACTF.Identity,
                                         bias=colGB_t[:, t:t + 1], scale=1.0)
                    nc.scalar.activation(out=gB[:], in_=gB[:], func=ACTF.Identity,
                                         bias=sc_t[:, 7:8], scale=1.0)
                    dcA = ap_.tile([128, W], F32, tag="d1cA", bufs=1)
                    dcB = ap_.tile([128, W], F32, tag="d1cB", bufs=1)
                    nc.vector.tensor_tensor(out=dcA[:], in0=d2a[:], in1=gA[:], op=OP.mult)
                    nc.vector.tensor_tensor(out=dcB[:], in0=d2b[:], in1=gB[:], op=OP.mult)
                    nc.scalar.activation(out=dcA[:], in_=dcA[:], func=ACTF.Identity,
                                         bias=sc_t[:, 3:4], scale=1.0)
                    nc.scalar.activation(out=dcB[:], in_=dcB[:], func=ACTF.Identity,
                                         bias=sc_t[:, 3:4], scale=1.0)
                    jq = ap_.tile([128, WP, 4], F32, tag="jq", bufs=1)
                    # cA(c) = c-1 for c=1..W ; cB(c) = c for c=0..W-1
                    nc.vector.tensor_copy(out=jq[:, 1:WP, 0], in_=dcA[:, 0:W])
                    nc.scalar.copy(out=jq[:, 1:WP, 1], in_=dcB[:, 0:W])
                    nc.vector.tensor_copy(out=jq[:, 0:W, 2], in_=dcA[:, 0:W])
                    nc.scalar.copy(out=jq[:, 0:W, 3], in_=dcB[:, 0:W])
                    # border columns: c=0 taps 0/1 (cA=0), c=W taps 2/3 (cB=W-1)
                    nc.vector.tensor_copy(out=jq[:, 0:1, 0], in_=dcA[:, 0:1])
                    nc.vector.tensor_copy(out=jq[:, 0:1, 1], in_=dcB[:, 0:1])
                    nc.vector.tensor_copy(out=jq[:, WP - 1:WP, 2], in_=dcA[:, W - 1:W])
                    nc.vector.tensor_copy(out=jq[:, WP - 1:WP, 3], in_=dcB[:, W - 1:W])
                    nc.sync.dma_start(out=J2S[y0 + 1:y0 + 129, :, :], in_=jq[:])

                for t in range(NTILES):
                    phase_a(t)
                nc.sync.dma_start(out=J2[:, :, :], in_=J2S[:])

                # ---- Phase B ----
                def ts(dst, in0, s1, s2, o0, o1=None):
                    nc.vector.tensor_scalar(out=dst, in0=in0, scalar1=s1,
                                            scalar2=s2, op0=o0,
                                            **({"op1": o1} if o1 is not None else {}))

                pending = None  # (weights..., gq) awaiting combine

                def coords_and_gather(rt, ct, z1):
                    y0 = 128 * rt
                    c0 = WT * ct
                    z1v = z1[:, c0:c0 + WT]
                    # affine terms (ACT: row + per-partition col const)
                    At = tp.tile([128, WT], F32, tag="fa")
                    Bt = tp.tile([128, WT], F32, tag="fb")
                    Ct = tp.tile([128, WT], F32, tag="fc")
                    nc.scalar.activation(out=At[:], in_=rowA_t[:, c0:c0 + WT],
                                         func=ACTF.Identity, bias=colA_t[:, rt:rt + 1], scale=1.0)
                    nc.scalar.activation(out=At[:], in_=At[:],
                                         func=ACTF.Identity, bias=sc_t[:, 4:5], scale=1.0)
                    nc.scalar.activation(out=Bt[:], in_=rowB_t[:, c0:c0 + WT],
                                         func=ACTF.Identity, bias=colB_t[:, rt:rt + 1], scale=1.0)
                    nc.scalar.activation(out=Bt[:], in_=Bt[:],
                                         func=ACTF.Identity, bias=sc_t[:, 5:6], scale=1.0)
                    nc.scalar.activation(out=Ct[:], in_=rowC_t[:, c0:c0 + WT],
                                         func=ACTF.Identity, bias=colC_t[:, rt:rt + 1], scale=1.0)
                    nc.scalar.activation(out=Ct[:], in_=Ct[:],
                                         func=ACTF.Identity, bias=sc_t[:, 6:7], scale=1.0)
                    # z2 = z1*C + Wv2 ; q = -1/z2 (recip + 1 Newton, sign folded)
                    z2 = tp.tile([128, WT], F32, tag="fd")
                    nc.vector.tensor_tensor(out=z2[:], in0=z1v, in1=Ct[:], op=OP.mult)
                    nc.scalar.activation(out=z2[:], in_=z2[:], func=ACTF.Identity,
                                         bias=sc_t[:, 2:3], scale=1.0)
                    r0 = tp.tile([128, WT], F32, tag="fe")
                    nc.vector.reciprocal(out=r0[:], in_=z2[:])
                    e = tp.tile([128, WT], F32, tag="ff")
                    nc.vector.tensor_tensor(out=e[:], in0=z2[:], in1=r0[:], op=OP.mult)
                    ts(e[:], e[:], 2.0, None, OP.subtract)            # z2*r0 - 2
                    q = tp.tile([128, WT], F32, tag="fq")
                    nc.vector.tensor_tensor(out=q[:], in0=r0[:], in1=e[:], op=OP.mult)  # ~ -1/z2
                    # u2 = (z1*(-A) + (-Wv0)) * q ; v2 likewise
                    nU = tp.tile([128, WT], F32, tag="fn")
                    nc.vector.tensor_tensor(out=nU[:], in0=z1v, in1=At[:], op=OP.mult)
                    nc.scalar.activation(out=nU[:], in_=nU[:], func=ACTF.Identity,
                                         bias=sc_t[:, 0:1], scale=1.0)
                    u2 = tp.tile([128, WT], F32, tag="fu")
                    nc.vector.tensor_tensor(out=u2[:], in0=nU[:], in1=q[:], op=OP.mult)
                    # Markstein correction: e = z2*u2' + nU ; u2 = u2' + e*q
                    # (keeps u2 within 0.5ulp of the reference's IEEE divide,
                    # which matters at the clip-edge discontinuities)
                    nc.vector.tensor_tensor(out=e[:], in0=z2[:], in1=u2[:], op=OP.mult)
                    nc.vector.tensor_tensor(out=e[:], in0=e[:], in1=nU[:], op=OP.add)
                    nc.vector.tensor_tensor(out=e[:], in0=e[:], in1=q[:], op=OP.mult)
                    nc.vector.tensor_tensor(out=u2[:], in0=u2[:], in1=e[:], op=OP.add)
                    nc.vector.tensor_tensor(out=nU[:], in0=z1v, in1=Bt[:], op=OP.mult)
                    nc.scalar.activation(out=nU[:], in_=nU[:], func=ACTF.Identity,
                                         bias=sc_t[:, 1:2], scale=1.0)
                    v2 = tp.tile([128, WT], F32, tag="fv")
                    nc.vector.tensor_tensor(out=v2[:], in0=nU[:], in1=q[:], op=OP.mult)
                    nc.vector.tensor_tensor(out=e[:], in0=z2[:], in1=v2[:], op=OP.mult)
                    nc.vector.tensor_tensor(out=e[:], in0=e[:], in1=nU[:], op=OP.add)
                    nc.vector.tensor_tensor(out=e[:], in0=e[:], in1=q[:], op=OP.mult)
                    nc.vector.tensor_tensor(out=v2[:], in0=v2[:], in1=e[:], op=OP.add)

                    # exact floor in f32: t = RNE(x) via magic add/sub (two
                    # separate instructions so the intermediate rounds to
                    # f32), then t -= (t > x). No convert-mode dependence.
                    xf = tp.tile([128, WT], F32, tag="xf")
                    yf = tp.tile([128, WT], F32, tag="yf")
                    gt = tp.tile([128, WT], F32, tag="gtf")
                    ts(xf[:], u2[:], MAGIC, None, OP.add)
                    ts(xf[:], xf[:], MAGIC, None, OP.subtract)
                    nc.vector.tensor_tensor(out=gt[:], in0=xf[:], in1=u2[:], op=OP.is_gt)
                    nc.vector.tensor_tensor(out=xf[:], in0=xf[:], in1=gt[:], op=OP.subtract)
                    ts(yf[:], v2[:], MAGIC, None, OP.add)
                    ts(yf[:], yf[:], MAGIC, None, OP.subtract)
                    nc.vector.tensor_tensor(out=gt[:], in0=yf[:], in1=v2[:], op=OP.is_gt)
                    nc.vector.tensor_tensor(out=yf[:], in0=yf[:], in1=gt[:], op=OP.subtract)
                    # clips (float domain; all values integer-exact)
                    x1p = tp.tile([128, WT], F32, tag="x1p")
                    ccf = tp.tile([128, WT], F32, tag="ccf")
                    y1p = tp.tile([128, WT], F32, tag="y1p")
                    rrf = tp.tile([128, WT], F32, tag="rrf")
                    ts(x1p[:], xf[:], 1.0, 0.0, OP.add, OP.max)
                    ts(ccf[:], x1p[:], float(W), None, OP.min)
                    ts(y1p[:], yf[:], 1.0, 0.0, OP.add, OP.max)
                    ts(rrf[:], y1p[:], float(H), None, OP.min)
                    # weights tiles (double-buffered across the pipeline)
                    wxa = pp.tile([128, WT], F32, tag="wxa")
                    wxc = pp.tile([128, WT], F32, tag="wxc")
                    wya = pp.tile([128, WT], F32, tag="wya")
                    wyb = pp.tile([128, WT], F32, tag="wyb")
                    # x0f = clip(xf,0,W-1), x1f = min(x1p,W-1) (reuse xf/x1p)
                    ts(xf[:], xf[:], 0.0, float(W - 1), OP.max, OP.min)
                    ts(x1p[:], x1p[:], float(W - 1), None, OP.min)
                    ts(yf[:], yf[:], 0.0, float(H - 1), OP.max, OP.min)
                    ts(y1p[:], y1p[:], float(H - 1), None, OP.min)
                    nc.vector.tensor_tensor(out=wxa[:], in0=x1p[:], in1=u2[:], op=OP.subtract)
                    nc.vector.tensor_tensor(out=wxc[:], in0=u2[:], in1=xf[:], op=OP.subtract)
                    nc.vector.tensor_tensor(out=wya[:], in0=y1p[:], in1=v2[:], op=OP.subtract)
                    nc.vector.tensor_tensor(out=wyb[:], in0=v2[:], in1=yf[:], op=OP.subtract)
                    # flat index = rr*WP + cc (exact in f32), one i32 convert
                    flatf = tp.tile([128, WT], F32, tag="flatf")
                    nc.vector.scalar_tensor_tensor(
                        out=flatf[:], in0=rrf[:], scalar=float(WP), in1=ccf[:],
                        op0=OP.mult, op1=OP.add)
                    flat = pp.tile([128, WT], I32, tag="flat")
                    nc.vector.tensor_copy(out=flat[:], in_=flatf[:])

                    # single-offset indirect gathers: 128 quads (one column
                    # of the col-tile) per instruction, fully unrolled so the
                    # Pool stream is pure INDIRECT1D (no For_i all-engine
                    # barriers / dge-drains), 4 SWDGE queues round-robin
                    gq = gp.tile([128, WT, 4], F32, tag="gq")
                    for j in range(WT):
                        inst = nc.gpsimd.indirect_dma_start(
                            out=gq[:, j, :], out_offset=None,
                            in_=J2flat,
                            in_offset=bass.IndirectOffsetOnAxis(ap=flat[:, j:j + 1], axis=0),
                        )
                        inst.ins.queue = f"qPoolDynamic{j % 4 or ''}"
                    return (rt, ct, wxa, wxc, wya, wyb, gq)

                def combine(p):
                    rt, ct, wxa, wxc, wya, wyb, gq = p
                    y0 = 128 * rt
                    c0 = WT * ct
                    # one consolidated read of gq: the NEXT epoch's 640
                    # gather instructions then WAR-wait on this single
                    # copy instead of on 6 combine readers each.
                    gqc = tp.tile([128, WT, 4], F32, tag="gqc")
                    nc.scalar.copy(out=gqc[:], in_=gq[:])
                    px0 = tp.tile([128, WT], F32, tag="fa")
                    px1 = tp.tile([128, WT], F32, tag="fb")
                    tt = tp.tile([128, WT], F32, tag="fc")
                    nc.vector.tensor_tensor(out=px0[:], in0=wxa[:], in1=gqc[:, :, 0], op=OP.mult)
                    nc.vector.tensor_tensor(out=tt[:], in0=wxc[:], in1=gqc[:, :, 2], op=OP.mult)
                    nc.vector.tensor_tensor(out=px0[:], in0=px0[:], in1=tt[:], op=OP.add)
                    nc.vector.tensor_tensor(out=px1[:], in0=wxa[:], in1=gqc[:, :, 1], op=OP.mult)
                    nc.vector.tensor_tensor(out=tt[:], in0=wxc[:], in1=gqc[:, :, 3], op=OP.mult)
                    nc.vector.tensor_tensor(out=px1[:], in0=px1[:], in1=tt[:], op=OP.add)
                    o = iop.tile([128, WT], F32, tag="ot")
                    nc.vector.tensor_tensor(out=o[:], in0=wya[:], in1=px0[:], op=OP.mult)
                    nc.vector.tensor_tensor(out=tt[:], in0=wyb[:], in1=px1[:], op=OP.mult)
                    nc.vector.tensor_tensor(out=o[:], in0=o[:], in1=tt[:], op=OP.add)
                    nc.sync.dma_start(out=out[lb, y0:y0 + 128, c0:c0 + WT], in_=o[:])

                for rt in range(NTILES):
                    z1 = iop.tile([128, W], F32, tag="z1")
                    nc.sync.dma_start(out=z1[:], in_=d1[lb, 128 * rt:128 * rt + 128, :])
                    for ct in range(NCT):
                        cur = coords_and_gather(rt, ct, z1)
                        if pending is not None:
                            combine(pending)
                        pending = cur
                combine(pending)

    nc.finalize()
    return nc


def _host_aux(translation, rotation, intrinsic):
    """Per-batch coefficient tensors (f32, mirroring reference order)."""
    K = intrinsic.astype(np.float32)
    Kinv = np.linalg.inv(K).astype(np.float32)
    R = rotation.astype(np.float32)
    t = translation.astype(np.float32)
    nb = R.shape[0]
    temp = np.einsum('ij,bkj->bik', K, R).astype(np.float32)
    Wv = np.einsum('bij,bjk->bik', temp, -t).astype(np.float32)     # [nb,3,1]
    M = np.einsum('bij,jk->bik', temp, Kinv).astype(np.float32)     # [nb,3,3]
    W2 = np.einsum('ij,bjk->bik', K, t).astype(np.float32)
    M2 = np.einsum('bij,jk->bik', np.einsum('ij,bjk->bik', K, R), Kinv).astype(np.float32)

    x = np.arange(W, dtype=np.float32)
    y = np.arange(H, dtype=np.float32)
    ycols = y.reshape(NTILES, 128).T                                # [128, NTILES]

    def rep_row(v):     # [nb, W] -> [nb, 128, W]
        return np.repeat(v[:, None, :], 128, axis=1).astype(np.float32)

    aux = {}
    aux["rowA"] = rep_row(-(M[:, 0, 0][:, None] * x[None, :]))
    aux["rowB"] = rep_row(-(M[:, 1, 0][:, None] * x[None, :]))
    aux["rowC"] = rep_row(M[:, 2, 0][:, None] * x[None, :])
    aux["rowG"] = rep_row(M2[:, 2, 0][:, None] * x[None, :])
    aux["colA"] = -(M[:, 0, 1][:, None, None] * ycols[None])
    aux["colB"] = -(M[:, 1, 1][:, None, None] * ycols[None])
    aux["colC"] = (M[:, 2, 1][:, None, None] * ycols[None])
    aux["colG"] = (M2[:, 2, 1][:, None, None] * ycols[None])
    ycolsB = np.minimum(ycols + 1.0, float(H - 1)).astype(np.float32)
    aux["colGB"] = (M2[:, 2, 1][:, None, None] * ycolsB[None])
    sc = np.stack([-Wv[:, 0, 0], -Wv[:, 1, 0], Wv[:, 2, 0], W2[:, 2, 0],
                   -M[:, 0, 2], -M[:, 1, 2], M[:, 2, 2], M2[:, 2, 2]], axis=1)
    aux["sc"] = np.repeat(sc[:, None, :], 128, axis=1)              # [nb,128,8]
    for k in aux:
        aux[k] = np.ascontiguousarray(aux[k].astype(np.float32))
    return aux


_NC_CACHE = {}


def kernel(depth_map_1, depth_map_2, translation, rotation, intrinsic):
    d1 = np.ascontiguousarray(np.asarray(depth_map_1, dtype=np.float32)[..., 0])
    d2 = np.ascontiguousarray(np.asarray(depth_map_2, dtype=np.float32)[..., 0])
    t = np.asarray(translation, dtype=np.float32)
    R = np.asarray(rotation, dtype=np.float32)
    K = np.asarray(intrinsic, dtype=np.float32)

    if "nc" not in _NC_CACHE:
        _NC_CACHE["nc"] = _build_bass()
    nc = _NC_CACHE["nc"]

    aux = _host_aux(t, R, K)

    in_maps = []
    for c in range(NCORES):
        sl = slice(c * BPC, (c + 1) * BPC)
        m = {"d1": d1[sl], "d2": d2[sl]}
        for k, v in aux.items():
            m[k] = v[sl]
        in_maps.append(m)

    res = bass_utils.run_bass_kernel_spmd(nc, in_maps, core_ids=list(range(NCORES)))
    out = np.empty((B, H, W, 1), np.float32)
    for c in range(NCORES):
        out[c * BPC:(c + 1) * BPC, :, :, 0] = res.results[c]["out"]
    return out


# revision 27
# speedup vs baseline: 1.1760x; 1.0073x over previous
"""Depth-warping layer for Trainium2 (Bass/Tile), 8-core data-parallel.

Strategy (v2)
-------------
Pure data parallelism over batch: each of 8 NeuronCores processes 2 of
the 16 images end to end (no collectives).

Per image:
  Phase A: build a "quad table" J2[(H+1) x (W+1), 4] in device DRAM,
    J2[r, c] = (I[rA,cA], I[rB,cA], I[rA,cB], I[rB,cB]) with
    rA=r-1, rB=min(r,H-1), cA=c-1, cB=min(c,W-1) (rows r>=1, cols
    1..W; row 0 is zeros and cols 0/W carry pairwise-equal taps so the
    clipped-border bilinear weights cancel exactly, matching the
    reference's clip semantics). I = d1_calc = W2z + d2*(affine in x,y).
    Built fully in SBUF per 128-row tile (strided interleave copies,
    row+1 tap via an SBUF partition-shift DMA), then ONE contiguous
    2.6MB DMA store per tile — no strided DRAM writes.
  Phase B: coordinate math in f32 only (floor via the 1.5*2^23
    round-to-nearest trick, so no dependence on the f32->i32 convert
    rounding mode), one i32 convert for the flattened gather index,
    then per-pixel 16B-quad gathers via single-offset-per-partition
    indirect DMA (128 descriptors/instruction — the only form the
    SWDGE vector-DGE executes correctly; multi-offset APs scramble),
    fully unrolled (no tc.For_i: its per-iteration all-engine barrier
    +dge-drain costs ~15%) over 4 SWDGE queues, and a factored 9-op
    bilinear combine.

The gather is the wall: one 128-descriptor Pool instruction per 128
pixels at ~1.1us fixed SWDGE cost each, so everything else is arranged
to hide completely under the Pool engine's instruction stream.

Host does only the per-batch 3x3 matrix algebra and ships per-batch
row/column/scalar coefficient tensors (the NEFF is shared by all cores,
so per-batch constants arrive as data).
"""

import numpy as np

import concourse.bass as bass
import concourse.bacc as bacc
import concourse.mybir as mybir
from concourse.tile import TileContext
from concourse import bass_utils

B, H, W = 16, 1024, 1280
NCORES = 8
BPC = B // NCORES          # batches per core
HP = H + 1                 # J2 rows
WP = W + 1                 # J2 cols
NTILES = H // 128
WT = 640                   # phase-B column tile (2 per row tile)
NCT = W // WT
UNROLL = 64                # gathers per For_i iteration

F32 = mybir.dt.float32
I32 = mybir.dt.int32
OP = mybir.AluOpType
ACTF = mybir.ActivationFunctionType

MAGIC = 12582912.0         # 1.5 * 2^23: forces round-to-int in f32 add


def _build_bass():
    nc = bacc.Bacc(target_bir_lowering=False, num_swdge_queues=1)

    d1 = nc.dram_tensor("d1", [BPC, H, W], F32, kind="ExternalInput")
    d2 = nc.dram_tensor("d2", [BPC, H, W], F32, kind="ExternalInput")
    # Replicated row-planes [BPC, 128, W] (DVE/ACT cannot partition-
    # broadcast, so host replicates). rowA/rowB are NEGATED on host
    # (-M00*x, -M10*x) because q approximates -1/z2.
    rowA = nc.dram_tensor("rowA", [BPC, 128, W], F32, kind="ExternalInput")
    rowB = nc.dram_tensor("rowB", [BPC, 128, W], F32, kind="ExternalInput")
    rowC = nc.dram_tensor("rowC", [BPC, 128, W], F32, kind="ExternalInput")
    rowG = nc.dram_tensor("rowG", [BPC, 128, W], F32, kind="ExternalInput")
    # Per-tile per-partition columns [BPC, 128, NTILES]; constants folded:
    # colA = -(M01*y + M02), colB = -(M11*y + M12), colC = M21*y + M22,
    # colG = M2_21*y + M2_22.
    colA = nc.dram_tensor("colA", [BPC, 128, NTILES], F32, kind="ExternalInput")
    colB = nc.dram_tensor("colB", [BPC, 128, NTILES], F32, kind="ExternalInput")
    colC = nc.dram_tensor("colC", [BPC, 128, NTILES], F32, kind="ExternalInput")
    colG = nc.dram_tensor("colG", [BPC, 128, NTILES], F32, kind="ExternalInput")
    colGB = nc.dram_tensor("colGB", [BPC, 128, NTILES], F32, kind="ExternalInput")
    # Per-batch scalars [BPC, 128, 8]:
    # (-Wv0, -Wv1, +Wv2, +W2z, -M02, -M12, +M22, +M2_22)
    sc = nc.dram_tensor("sc", [BPC, 128, 8], F32, kind="ExternalInput")
    out = nc.dram_tensor("out", [BPC, H, W], F32, kind="ExternalOutput")

    with TileContext(nc) as tc:
        with tc.tile_pool(name="dram", bufs=2, space="DRAM") as dpool, \
             tc.tile_pool(name="cst", bufs=1) as cpool, \
             tc.tile_pool(name="io", bufs=2) as iop, \
             tc.tile_pool(name="ph_a", bufs=2) as ap_, \
             tc.tile_pool(name="tmp", bufs=1) as tp, \
             tc.tile_pool(name="pipe", bufs=2) as pp, \
             tc.tile_pool(name="gat", bufs=2) as gp:

            # zeros for J2 row 0 (4 partitions x WP f32 = one 20.5KB row)
            zrow = cpool.tile([4, WP], F32)
            nc.vector.memset(zrow[:], 0.0)

            for lb in range(BPC):
                # staging table: phase A writes land here (9 writer
                # instructions); ONE bulk copy then produces J2, so every
                # gather instruction carries a single J2-writer sem wait
                # instead of nine (the SEQ decodes each wait per gather).
                J2S = dpool.tile([HP, WP, 4], F32, tag="J2S")
                J2 = dpool.tile([HP, WP, 4], F32, tag="J2")
                J2flat = J2[:].rearrange("a b c -> (a b) c")

                # ---- per-batch coefficient loads ----
                sc_t = cpool.tile([128, 8], F32, tag="sc")
                nc.sync.dma_start(out=sc_t[:], in_=sc[lb])
                rowA_t = cpool.tile([128, W], F32, tag="rowA")
                rowB_t = cpool.tile([128, W], F32, tag="rowB")
                rowC_t = cpool.tile([128, W], F32, tag="rowC")
                rowG_t = cpool.tile([128, W], F32, tag="rowG")
                nc.sync.dma_start(out=rowA_t[:], in_=rowA[lb])
                nc.sync.dma_start(out=rowB_t[:], in_=rowB[lb])
                nc.sync.dma_start(out=rowC_t[:], in_=rowC[lb])
                nc.sync.dma_start(out=rowG_t[:], in_=rowG[lb])
                colA_t = cpool.tile([128, NTILES], F32, tag="colA")
                colB_t = cpool.tile([128, NTILES], F32, tag="colB")
                colC_t = cpool.tile([128, NTILES], F32, tag="colC")
                colG_t = cpool.tile([128, NTILES], F32, tag="colG")
                colGB_t = cpool.tile([128, NTILES], F32, tag="colGB")
                nc.sync.dma_start(out=colA_t[:], in_=colA[lb])
                nc.sync.dma_start(out=colB_t[:], in_=colB[lb])
                nc.sync.dma_start(out=colC_t[:], in_=colC[lb])
                nc.sync.dma_start(out=colG_t[:], in_=colG[lb])
                nc.sync.dma_start(out=colGB_t[:], in_=colGB[lb])

                # ---- Phase A: build J2 (into J2S) ----
                # row 0 = zeros (border row; weights cancel exactly)
                nc.sync.dma_start(out=J2S[0:1, :, :], in_=zrow[:])

                def phase_a(t):
                    """Build J2 rows [128t+1, 128t+129): A-tap = image row
                    128t+p, B-tap = image row min(128t+p+1, H-1)."""
                    y0 = 128 * t
                    d2a = iop.tile([128, W], F32, tag="d2a")
                    d2b = iop.tile([128, W], F32, tag="d2b")
                    nc.sync.dma_start(out=d2a[:], in_=d2[lb, y0:y0 + 128, :])
                    if t < NTILES - 1:
                        nc.sync.dma_start(out=d2b[:], in_=d2[lb, y0 + 1:y0 + 129, :])
                    else:
                        nc.sync.dma_start(out=d2b[0:127, :], in_=d2[lb, y0 + 1:y0 + 128, :])
                        nc.sync.dma_start(out=d2b[127:128, :], in_=d2[lb, H - 1:H, :])
                    gA = ap_.tile([128, W], F32, tag="gA", bufs=1)
                    gB = ap_.tile([128, W], F32, tag="gB", bufs=1)
                    nc.scalar.activation(out=gA[:], in_=rowG_t[:], func=ACTF.Identity,
                                         bias=colG_t[:, t:t + 1], scale=1.0)
                    nc.scalar.activation(out=gA[:], in_=gA[:], func=ACTF.Identity,
                                         bias=sc_t[:, 7:8], scale=1.0)
                    nc.scalar.activation(out=gB[:], in_=rowG_t[:], func=ACTF.Identity,
                                         bias=colGB_t[:, t:t + 1], scale=1.0)
                    nc.scalar.activation(out=gB[:], in_=gB[:], func=ACTF.Identity,
                                         bias=sc_t[:, 7:8], scale=1.0)
                    dcA = ap_.tile([128, W], F32, tag="d1cA", bufs=1)
                    dcB = ap_.tile([128, W], F32, tag="d1cB", bufs=1)
                    nc.vector.tensor_tensor(out=dcA[:], in0=d2a[:], in1=gA[:], op=OP.mult)
                    nc.vector.tensor_tensor(out=dcB[:], in0=d2b[:], in1=gB[:], op=OP.mult)
                    nc.scalar.activation(out=dcA[:], in_=dcA[:], func=ACTF.Identity,
                                         bias=sc_t[:, 3:4], scale=1.0)
                    nc.scalar.activation(out=dcB[:], in_=dcB[:], func=ACTF.Identity,
                                         bias=sc_t[:, 3:4], scale=1.0)
                    jq = ap_.tile([128, WP, 4], F32, tag="jq", bufs=1)
                    # cA(c) = c-1 for c=1..W ; cB(c) = c for c=0..W-1
                    nc.vector.tensor_copy(out=jq[:, 1:WP, 0], in_=dcA[:, 0:W])
                    nc.scalar.copy(out=jq[:, 1:WP, 1], in_=dcB[:, 0:W])
                    nc.vector.tensor_copy(out=jq[:, 0:W, 2], in_=dcA[:, 0:W])
                    nc.scalar.copy(out=jq[:, 0:W, 3], in_=dcB[:, 0:W])
                    # border columns: c=0 taps 0/1 (cA=0), c=W taps 2/3 (cB=W-1)
                    nc.vector.tensor_copy(out=jq[:, 0:1, 0], in_=dcA[:, 0:1])
                    nc.vector.tensor_copy(out=jq[:, 0:1, 1], in_=dcB[:, 0:1])
                    nc.vector.tensor_copy(out=jq[:, WP - 1:WP, 2], in_=dcA[:, W - 1:W])
                    nc.vector.tensor_copy(out=jq[:, WP - 1:WP, 3], in_=dcB[:, W - 1:W])
                    nc.sync.dma_start(out=J2S[y0 + 1:y0 + 129, :, :], in_=jq[:])

                for t in range(NTILES):
                    phase_a(t)
                nc.sync.dma_start(out=J2[:, :, :], in_=J2S[:])

                # ---- Phase B ----
                def ts(dst, in0, s1, s2, o0, o1=None):
                    nc.vector.tensor_scalar(out=dst, in0=in0, scalar1=s1,
                                            scalar2=s2, op0=o0,
                                            **({"op1": o1} if o1 is not None else {}))

                pending = None  # (weights..., gq) awaiting combine

                def coords_and_gather(rt, ct, z1):
                    y0 = 128 * rt
                    c0 = WT * ct
                    z1v = z1[:, c0:c0 + WT]
                    # affine terms (ACT: row + per-partition col const)
                    At = tp.tile([128, WT], F32, tag="fa")
                    Bt = tp.tile([128, WT], F32, tag="fb")
                    Ct = tp.tile([128, WT], F32, tag="fc")
                    nc.scalar.activation(out=At[:], in_=rowA_t[:, c0:c0 + WT],
                                         func=ACTF.Identity, bias=colA_t[:, rt:rt + 1], scale=1.0)
                    nc.scalar.activation(out=At[:], in_=At[:],
                                         func=ACTF.Identity, bias=sc_t[:, 4:5], scale=1.0)
                    nc.scalar.activation(out=Bt[:], in_=rowB_t[:, c0:c0 + WT],
                                         func=ACTF.Identity, bias=colB_t[:, rt:rt + 1], scale=1.0)
                    nc.scalar.activation(out=Bt[:], in_=Bt[:],
                                         func=ACTF.Identity, bias=sc_t[:, 5:6], scale=1.0)
                    nc.scalar.activation(out=Ct[:], in_=rowC_t[:, c0:c0 + WT],
                                         func=ACTF.Identity, bias=colC_t[:, rt:rt + 1], scale=1.0)
                    nc.scalar.activation(out=Ct[:], in_=Ct[:],
                                         func=ACTF.Identity, bias=sc_t[:, 6:7], scale=1.0)
                    # z2 = z1*C + Wv2 ; q = -1/z2 (recip + 1 Newton, sign folded)
                    z2 = tp.tile([128, WT], F32, tag="fd")
                    nc.vector.tensor_tensor(out=z2[:], in0=z1v, in1=Ct[:], op=OP.mult)
                    nc.scalar.activation(out=z2[:], in_=z2[:], func=ACTF.Identity,
                                         bias=sc_t[:, 2:3], scale=1.0)
                    r0 = tp.tile([128, WT], F32, tag="fe")
                    nc.vector.reciprocal(out=r0[:], in_=z2[:])
                    e = tp.tile([128, WT], F32, tag="ff")
                    nc.vector.tensor_tensor(out=e[:], in0=z2[:], in1=r0[:], op=OP.mult)
                    ts(e[:], e[:], 2.0, None, OP.subtract)            # z2*r0 - 2
                    q = tp.tile([128, WT], F32, tag="fq")
                    nc.vector.tensor_tensor(out=q[:], in0=r0[:], in1=e[:], op=OP.mult)  # ~ -1/z2
                    # u2 = (z1*(-A) + (-Wv0)) * q ; v2 likewise
                    nU = tp.tile([128, WT], F32, tag="fn")
                    nc.vector.tensor_tensor(out=nU[:], in0=z1v, in1=At[:], op=OP.mult)
                    nc.scalar.activation(out=nU[:], in_=nU[:], func=ACTF.Identity,
                                         bias=sc_t[:, 0:1], scale=1.0)
                    u2 = tp.tile([128, WT], F32, tag="fu")
                    nc.vector.tensor_tensor(out=u2[:], in0=nU[:], in1=q[:], op=OP.mult)
                    # Markstein correction: e = z2*u2' + nU ; u2 = u2' + e*q
                    # (keeps u2 within 0.5ulp of the reference's IEEE divide,
                    # which matters at the clip-edge discontinuities)
                    nc.vector.tensor_tensor(out=e[:], in0=z2[:], in1=u2[:], op=OP.mult)
                    nc.vector.tensor_tensor(out=e[:], in0=e[:], in1=nU[:], op=OP.add)
                    nc.vector.tensor_tensor(out=e[:], in0=e[:], in1=q[:], op=OP.mult)
                    nc.vector.tensor_tensor(out=u2[:], in0=u2[:], in1=e[:], op=OP.add)
                    nc.vector.tensor_tensor(out=nU[:], in0=z1v, in1=Bt[:], op=OP.mult)
                    nc.scalar.activation(out=nU[:], in_=nU[:], func=ACTF.Identity,
                                         bias=sc_t[:, 1:2], scale=1.0)
                    v2 = tp.tile([128, WT], F32, tag="fv")
                    nc.vector.tensor_tensor(out=v2[:], in0=nU[:], in1=q[:], op=OP.mult)
                    nc.vector.tensor_tensor(out=e[:], in0=z2[:], in1=v2[:], op=OP.mult)
                    nc.vector.tensor_tensor(out=e[:], in0=e[:], in1=nU[:], op=OP.add)
                    nc.vector.tensor_tensor(out=e[:], in0=e[:], in1=q[:], op=OP.mult)
                    nc.vector.tensor_tensor(out=v2[:], in0=v2[:], in1=e[:], op=OP.add)

                    # exact floor in f32: t = RNE(x) via magic add/sub (two
                    # separate instructions so the intermediate rounds to
                    # f32), then t -= (t > x). No convert-mode dependence.
                    xf = tp.tile([128, WT], F32, tag="xf")
                    yf = tp.tile([128, WT], F32, tag="yf")
                    gt = tp.tile([128, WT], F32, tag="gtf")
                    ts(xf[:], u2[:], MAGIC, None, OP.add)
                    ts(xf[:], xf[:], MAGIC, None, OP.subtract)
                    nc.vector.tensor_tensor(out=gt[:], in0=xf[:], in1=u2[:], op=OP.is_gt)
                    nc.vector.tensor_tensor(out=xf[:], in0=xf[:], in1=gt[:], op=OP.subtract)
                    ts(yf[:], v2[:], MAGIC, None, OP.add)
                    ts(yf[:], yf[:], MAGIC, None, OP.subtract)
                    nc.vector.tensor_tensor(out=gt[:], in0=yf[:], in1=v2[:], op=OP.is_gt)
                    nc.vector.tensor_tensor(out=yf[:], in0=yf[:], in1=gt[:], op=OP.subtract)
                    # clips (float domain; all values integer-exact)
                    x1p = tp.tile([128, WT], F32, tag="x1p")
                    ccf = tp.tile([128, WT], F32, tag="ccf")
                    y1p = tp.tile([128, WT], F32, tag="y1p")
                    rrf = tp.tile([128, WT], F32, tag="rrf")
                    ts(x1p[:], xf[:], 1.0, 0.0, OP.add, OP.max)
                    ts(ccf[:], x1p[:], float(W), None, OP.min)
                    ts(y1p[:], yf[:], 1.0, 0.0, OP.add, OP.max)
                    ts(rrf[:], y1p[:], float(H), None, OP.min)
                    # weights tiles (double-buffered across the pipeline)
                    wxa = pp.tile([128, WT], F32, tag="wxa")
                    wxc = pp.tile([128, WT], F32, tag="wxc")
                    wya = pp.tile([128, WT], F32, tag="wya")
                    wyb = pp.tile([128, WT], F32, tag="wyb")
                    # x0f = clip(xf,0,W-1), x1f = min(x1p,W-1) (reuse xf/x1p)
                    ts(xf[:], xf[:], 0.0, float(W - 1), OP.max, OP.min)
                    ts(x1p[:], x1p[:], float(W - 1), None, OP.min)
                    ts(yf[:], yf[:], 0.0, float(H - 1), OP.max, OP.min)
                    ts(y1p[:], y1p[:], float(H - 1), None, OP.min)
                    nc.vector.tensor_tensor(out=wxa[:], in0=x1p[:], in1=u2[:], op=OP.subtract)
                    nc.vector.tensor_tensor(out=wxc[:], in0=u2[:], in1=xf[:], op=OP.subtract)
                    nc.vector.tensor_tensor(out=wya[:], in0=y1p[:], in1=v2[:], op=OP.subtract)
                    nc.vector.tensor_tensor(out=wyb[:], in0=v2[:], in1=yf[:], op=OP.subtract)
                    # flat index = rr*WP + cc (exact in f32), one i32 convert
                    flatf = tp.tile([128, WT], F32, tag="flatf")
                    nc.vector.scalar_tensor_tensor(
                        out=flatf[:], in0=rrf[:], scalar=float(WP), in1=ccf[:],
                        op0=OP.mult, op1=OP.add)
                    flat = pp.tile([128, WT], I32, tag="flat")
                    nc.vector.tensor_copy(out=flat[:], in_=flatf[:])

                    # single-offset indirect gathers: 128 quads (one column
                    # of the col-tile) per instruction, fully unrolled so the
                    # Pool stream is pure INDIRECT1D (no For_i all-engine
                    # barriers / dge-drains), 4 SWDGE queues round-robin
                    gq = gp.tile([128, WT, 4], F32, tag="gq")
                    for j in range(WT):
                        inst = nc.gpsimd.indirect_dma_start(
                            out=gq[:, j, :], out_offset=None,
                            in_=J2flat,
                            in_offset=bass.IndirectOffsetOnAxis(ap=flat[:, j:j + 1], axis=0),
                        )
                        inst.ins.queue = "qPoolDynamic"
                    return (rt, ct, wxa, wxc, wya, wyb, gq)

                def combine(p):
                    rt, ct, wxa, wxc, wya, wyb, gq = p
                    y0 = 128 * rt
                    c0 = WT * ct
                    # one consolidated read of gq: the NEXT epoch's 640
                    # gather instructions then WAR-wait on this single
                    # copy instead of on 6 combine readers each.
                    gqc = tp.tile([128, WT, 4], F32, tag="gqc")
                    nc.scalar.copy(out=gqc[:], in_=gq[:])
                    px0 = tp.tile([128, WT], F32, tag="fa")
                    px1 = tp.tile([128, WT], F32, tag="fb")
                    tt = tp.tile([128, WT], F32, tag="fc")
                    nc.vector.tensor_tensor(out=px0[:], in0=wxa[:], in1=gqc[:, :, 0], op=OP.mult)
                    nc.vector.tensor_tensor(out=tt[:], in0=wxc[:], in1=gqc[:, :, 2], op=OP.mult)
                    nc.vector.tensor_tensor(out=px0[:], in0=px0[:], in1=tt[:], op=OP.add)
                    nc.vector.tensor_tensor(out=px1[:], in0=wxa[:], in1=gqc[:, :, 1], op=OP.mult)
                    nc.vector.tensor_tensor(out=tt[:], in0=wxc[:], in1=gqc[:, :, 3], op=OP.mult)
                    nc.vector.tensor_tensor(out=px1[:], in0=px1[:], in1=tt[:], op=OP.add)
                    o = iop.tile([128, WT], F32, tag="ot")
                    nc.vector.tensor_tensor(out=o[:], in0=wya[:], in1=px0[:], op=OP.mult)
                    nc.vector.tensor_tensor(out=tt[:], in0=wyb[:], in1=px1[:], op=OP.mult)
                    nc.vector.tensor_tensor(out=o[:], in0=o[:], in1=tt[:], op=OP.add)
                    nc.sync.dma_start(out=out[lb, y0:y0 + 128, c0:c0 + WT], in_=o[:])

                for rt in range(NTILES):
                    z1 = iop.tile([128, W], F32, tag="z1")
                    nc.sync.dma_start(out=z1[:], in_=d1[lb, 128 * rt:128 * rt + 128, :])
                    for ct in range(NCT):
                        cur = coords_and_gather(rt, ct, z1)
                        if pending is not None:
                            combine(pending)
                        pending = cur
                combine(pending)

    nc.finalize()
    return nc


def _host_aux(translation, rotation, intrinsic):
    """Per-batch coefficient tensors (f32, mirroring reference order)."""
    K = intrinsic.astype(np.float32)
    Kinv = np.linalg.inv(K).astype(np.float32)
    R = rotation.astype(np.float32)
    t = translation.astype(np.float32)
    nb = R.shape[0]
    temp = np.einsum('ij,bkj->bik', K, R).astype(np.float32)
    Wv = np.einsum('bij,bjk->bik', temp, -t).astype(np.float32)     # [nb,3,1]
    M = np.einsum('bij,jk->bik', temp, Kinv).astype(np.float32)     # [nb,3,3]
    W2 = np.einsum('ij,bjk->bik', K, t).astype(np.float32)
    M2 = np.einsum('bij,jk->bik', np.einsum('ij,bjk->bik', K, R), Kinv).astype(np.float32)

    x = np.arange(W, dtype=np.float32)
    y = np.arange(H, dtype=np.float32)
    ycols = y.reshape(NTILES, 128).T                                # [128, NTILES]

    def rep_row(v):     # [nb, W] -> [nb, 128, W]
        return np.repeat(v[:, None, :], 128, axis=1).astype(np.float32)

    aux = {}
    aux["rowA"] = rep_row(-(M[:, 0, 0][:, None] * x[None, :]))
    aux["rowB"] = rep_row(-(M[:, 1, 0][:, None] * x[None, :]))
    aux["rowC"] = rep_row(M[:, 2, 0][:, None] * x[None, :])
    aux["rowG"] = rep_row(M2[:, 2, 0][:, None] * x[None, :])
    aux["colA"] = -(M[:, 0, 1][:, None, None] * ycols[None])
    aux["colB"] = -(M[:, 1, 1][:, None, None] * ycols[None])
    aux["colC"] = (M[:, 2, 1][:, None, None] * ycols[None])
    aux["colG"] = (M2[:, 2, 1][:, None, None] * ycols[None])
    ycolsB = np.minimum(ycols + 1.0, float(H - 1)).astype(np.float32)
    aux["colGB"] = (M2[:, 2, 1][:, None, None] * ycolsB[None])
    sc = np.stack([-Wv[:, 0, 0], -Wv[:, 1, 0], Wv[:, 2, 0], W2[:, 2, 0],
                   -M[:, 0, 2], -M[:, 1, 2], M[:, 2, 2], M2[:, 2, 2]], axis=1)
    aux["sc"] = np.repeat(sc[:, None, :], 128, axis=1)              # [nb,128,8]
    for k in aux:
        aux[k] = np.ascontiguousarray(aux[k].astype(np.float32))
    return aux


_NC_CACHE = {}


def kernel(depth_map_1, depth_map_2, translation, rotation, intrinsic):
    d1 = np.ascontiguousarray(np.asarray(depth_map_1, dtype=np.float32)[..., 0])
    d2 = np.ascontiguousarray(np.asarray(depth_map_2, dtype=np.float32)[..., 0])
    t = np.asarray(translation, dtype=np.float32)
    R = np.asarray(rotation, dtype=np.float32)
    K = np.asarray(intrinsic, dtype=np.float32)

    if "nc" not in _NC_CACHE:
        _NC_CACHE["nc"] = _build_bass()
    nc = _NC_CACHE["nc"]

    aux = _host_aux(t, R, K)

    in_maps = []
    for c in range(NCORES):
        sl = slice(c * BPC, (c + 1) * BPC)
        m = {"d1": d1[sl], "d2": d2[sl]}
        for k, v in aux.items():
            m[k] = v[sl]
        in_maps.append(m)

    res = bass_utils.run_bass_kernel_spmd(nc, in_maps, core_ids=list(range(NCORES)))
    out = np.empty((B, H, W, 1), np.float32)
    for c in range(NCORES):
        out[c * BPC:(c + 1) * BPC, :, :, 0] = res.results[c]["out"]
    return out
